# revision 25
# baseline (speedup 1.0000x reference)
"""Trainium2 Bass kernel for a custom GRU (T=2048, B=64, IN=128, H=256).

Strategy
--------
Data-parallel over batch: 8 NeuronCores x 8 batch rows each, every core runs
the full sequential scan on its shard (the nonlinear recurrence blocks
sequence parallelism).

Per core, everything lives in a "gates/hidden on partitions" layout so the
elementwise tail uses all 128 SBUF partitions:

  phase A (per 64-step chunk): ig^T = W_ih^T.T @ x^T  (fp32 PE matmuls,
      N=512), biases folded in with a DVE tensor_scalar add, result kept in
      SBUF in the scan's column layout [128, t*48 + m*8 + b].
  scan step: 12 bf16 matmuls (W_hh^T tiles stationary, h^T moving, N=8)
      accumulate hgates into three separate PSUM banks (r / z / c) so the
      DVE/ACT tail on one bank never serializes against PE writes to the
      others.  Tail: sigmoid/tanh on ACT (one table set), fused
      (psum+bias)*r via scalar_tensor_tensor on DVE, h update on DVE.
      h is kept fp32 for the update/output; only the matmul operand is
      rounded to bf16 (measured end-to-end rel_l2 ~6.7e-4 vs fp32).

Host side does all transposes (numpy), so the device never needs a
transpose: inputs are shipped pre-transposed, the output is shipped back in
[128, T*16] layout and rearranged to [T, B, H] on the host.
"""

import sys

if "/opt/trn_rl_repo" not in sys.path:
    sys.path.insert(0, "/opt/trn_rl_repo")

import numpy as np
import ml_dtypes

T, B, IN, H = 2048, 64, 128, 256
G3 = 3 * H               # 768
NCORES = 8
BS = B // NCORES         # 8 batch rows per core
SW = 2 * BS              # 16 columns of h per timestep (2 blocks of 128 dims)
CH = 64                  # timesteps per chunk
SCAN_BF16 = True         # bf16 recurrent matmul (fp32 psum accumulate)
OPT_IDENT = True         # preload psum with ig_r/ig_z/bhh_c via identity matmul
OPT_POOL_OUT = True      # fp32 output copy on gpsimd instead of DVE
OPT_ZM = True            # h' = z*h + (1-z)*c with 1-z = sigmoid(-x); z*h off-chain
PHASEA_F32R = True      # igates matmul in float32r (fast fp32 PE mode)

_cache = {}


def _build(t_total):
    import concourse.bacc as bacc
    import concourse.mybir as mybir
    import concourse.tile as tile

    f32 = mybir.dt.float32
    xdt = mybir.dt.float32r if PHASEA_F32R else f32
    wdt = mybir.dt.bfloat16 if SCAN_BF16 else f32
    AO = mybir.AluOpType
    AF = mybir.ActivationFunctionType

    nchunk = t_total // CH
    nc = bacc.Bacc("TRN2", target_bir_lowering=False, debug=False,
                   num_devices=NCORES)

    xT_d = nc.dram_tensor("xT", [IN, t_total * BS], xdt, kind="ExternalInput")
    wihT_d = nc.dram_tensor("wihT", [IN, G3], xdt, kind="ExternalInput")
    whhT_d = nc.dram_tensor("whhT", [H, G3], wdt, kind="ExternalInput")
    igb_d = nc.dram_tensor("igb", [128, 6], f32, kind="ExternalInput")
    bhc_d = nc.dram_tensor("bhc", [128, 2], f32, kind="ExternalInput")
    bhcb_d = nc.dram_tensor("bhcb", [128, SW], f32, kind="ExternalInput")
    ident_d = nc.dram_tensor("ident", [128, 128], f32, kind="ExternalInput")
    h0f_d = nc.dram_tensor("h0f", [128, SW], f32, kind="ExternalInput")
    h0b_d = nc.dram_tensor("h0b", [128, SW], wdt, kind="ExternalInput")
    out_d = nc.dram_tensor("outT", [128, t_total * SW], f32,
                           kind="ExternalOutput")

    with tile.TileContext(nc) as tc:
        with (
            tc.tile_pool(name="consts", bufs=1) as consts,
            tc.tile_pool(name="xc", bufs=3) as xcp,
            tc.tile_pool(name="ig", bufs=2) as igp,
            tc.tile_pool(name="outc", bufs=2) as outp,
            tc.tile_pool(name="tails", bufs=3) as tp,
            tc.tile_pool(name="hbf", bufs=3) as hbp,
            tc.tile_pool(name="psA", bufs=2, space="PSUM") as psa,
            tc.tile_pool(name="psB", bufs=2, space="PSUM") as psb,
            tc.tile_pool(name="psC", bufs=2, space="PSUM") as psc,
            tc.tile_pool(name="psP", bufs=2, space="PSUM") as psp,
        ):
            wih = consts.tile([IN, G3], xdt, tag="wih")
            nc.sync.dma_start(wih[:], wihT_d[:])
            whh0 = consts.tile([128, G3], wdt, tag="whh0")
            whh1 = consts.tile([128, G3], wdt, tag="whh1")
            nc.sync.dma_start(whh0[:], whhT_d[0:128, :])
            nc.sync.dma_start(whh1[:], whhT_d[128:256, :])
            igb = consts.tile([128, 6], f32, tag="igb")
            nc.sync.dma_start(igb[:], igb_d[:])
            bhc = consts.tile([128, 2], f32, tag="bhc")
            nc.sync.dma_start(bhc[:], bhc_d[:])
            if OPT_IDENT:
                bhcb = consts.tile([128, SW], f32, tag="bhcb")
                nc.sync.dma_start(bhcb[:], bhcb_d[:])
                ident = consts.tile([128, 128], f32, tag="ident")
                nc.sync.dma_start(ident[:], ident_d[:])
            h0f = consts.tile([128, SW], f32, tag="h0f")
            nc.sync.dma_start(h0f[:], h0f_d[:])
            h0b = consts.tile([128, SW], wdt, tag="h0b")
            nc.sync.dma_start(h0b[:], h0b_d[:])

            mm = nc.tensor.matmul
            hbf_prev = h0b[:, :]     # bf16 (or fp32) h for matmul rhs
            hfp_prev = h0f[:, :]     # fp32 h for the update

            NP = 1              # bias-add split (keep DVE pieces small)

            def phase_a_mm(xc, m):
                """One M-tile of the igates precompute matmul."""
                pp = psp.tile([128, CH * BS], f32, tag="pp")
                mm(pp[:], wih[:, m * 128:(m + 1) * 128], xc[:],
                   start=True, stop=True)
                return pp

            def phase_a_bias(pp, ig3, m, j, on_act=False):
                """Bias add + psum->sbuf rearrange, one slice of timesteps."""
                pp3 = pp[:].rearrange("p (t b) -> p t b", b=BS)
                t0, t1 = j * (CH // NP), (j + 1) * (CH // NP)
                dst = ig3[:, t0:t1, m * BS:(m + 1) * BS]
                src = pp3[:, t0:t1, :]
                if on_act:
                    nc.scalar.activation(dst, src, AF.Identity,
                                         bias=igb[:, m:m + 1])
                else:
                    nc.vector.tensor_scalar_add(dst, src, igb[:, m:m + 1])

            def load_chunk(ci):
                xc = xcp.tile([IN, CH * BS], xdt, tag="xc")
                nc.sync.dma_start(xc[:],
                                  xT_d[:, ci * CH * BS:(ci + 1) * CH * BS])
                igt = igp.tile([128, CH * 48], f32, tag="ig")
                ig3 = igt[:].rearrange("p (t c) -> p t c", c=48)
                return xc, igt, ig3

            # chunk 0's igates run up front; each later chunk's igates are
            # interleaved into the previous chunk's scan (PE/DVE idle slots)
            xc_cur, igt, ig3_cur = load_chunk(0)
            for m in range(6):
                pp = phase_a_mm(xc_cur, m)
                for j in range(NP):
                    phase_a_bias(pp, ig3_cur, m, j)

            for ci in range(nchunk):
                if ci + 1 < nchunk:
                    xc_next, igt_next, ig3_next = load_chunk(ci + 1)
                    # emission schedule: step -> list of thunks
                    sched = {}
                    pp_of = {}
                    for m in range(6):
                        def mk_mm(m=m):
                            pp_of[m] = phase_a_mm(xc_next, m)
                        sched.setdefault(8 * (m + 1), []).append(mk_mm)
                        for j in range(NP):
                            def mk_bias(m=m, j=j):
                                phase_a_bias(pp_of[m], ig3_next, m, j,
                                             on_act=(m % 2 == 1))
                            sched.setdefault(8 * (m + 1) + 2 * j + 1,
                                             []).append(mk_bias)
                else:
                    sched = {}

                # ---- scan over the chunk ----
                oc = outp.tile([128, CH * SW], f32, tag="oc")
                for tl in range(CH):
                    base = tl * 48
                    pA = psa.tile([128, SW], f32, tag="pA")
                    pB = psb.tile([128, SW], f32, tag="pB")
                    pC = psc.tile([128, SW], f32, tag="pC")
                    hb0 = hbf_prev[:, 0:BS]
                    hb1 = hbf_prev[:, BS:SW]
                    if OPT_IDENT:
                        # preload psum banks via identity matmul so the
                        # weight matmuls accumulate straight onto ig / bias
                        mm(pA[:], ident[:], igt[:, base:base + 16],
                           start=True, stop=False)
                        mm(pB[:], ident[:], igt[:, base + 16:base + 32],
                           start=True, stop=False)
                        mm(pC[:], ident[:], bhcb[:], start=True, stop=False)
                        s0 = False
                    else:
                        s0 = True
                    # r gates first (c-path needs r), then c, then z
                    mm(pA[:, 0:BS], whh0[:, 0:128], hb0, start=s0, stop=False)
                    mm(pA[:, 0:BS], whh1[:, 0:128], hb1, start=False, stop=False)
                    mm(pA[:, BS:SW], whh0[:, 128:256], hb0, start=False, stop=False)
                    mm(pA[:, BS:SW], whh1[:, 128:256], hb1, start=False, stop=True)
                    mm(pC[:, 0:BS], whh0[:, 512:640], hb0, start=s0, stop=False)
                    mm(pC[:, 0:BS], whh1[:, 512:640], hb1, start=False, stop=False)
                    mm(pC[:, BS:SW], whh0[:, 640:768], hb0, start=False, stop=False)
                    mm(pC[:, BS:SW], whh1[:, 640:768], hb1, start=False, stop=True)
                    mm(pB[:, 0:BS], whh0[:, 256:384], hb0, start=s0, stop=False)
                    mm(pB[:, 0:BS], whh1[:, 256:384], hb1, start=False, stop=False)
                    mm(pB[:, BS:SW], whh0[:, 384:512], hb0, start=False, stop=False)
                    mm(pB[:, BS:SW], whh1[:, 384:512], hb1, start=False, stop=True)

                    r = tp.tile([128, SW], f32, tag="r")
                    if OPT_IDENT:
                        # pA already holds ig_r + hg_r
                        nc.scalar.activation(r[:], pA[:], AF.Sigmoid)
                        tmpc = tp.tile([128, SW], f32, tag="tmpc")
                        nc.vector.tensor_tensor(tmpc[:], pC[:], r[:],
                                                op=AO.mult)
                    else:
                        nc.vector.tensor_tensor(pA[:], pA[:],
                                                igt[:, base:base + 16],
                                                op=AO.add)
                        nc.scalar.activation(r[:], pA[:], AF.Sigmoid)
                        # tmpc = (hg_c + b_hh_c) * r   (per-128-block bias)
                        tmpc = tp.tile([128, SW], f32, tag="tmpc")
                        nc.vector.scalar_tensor_tensor(
                            tmpc[:, 0:BS], pC[:, 0:BS], bhc[:, 0:1], r[:, 0:BS],
                            op0=AO.add, op1=AO.mult)
                        nc.vector.scalar_tensor_tensor(
                            tmpc[:, BS:SW], pC[:, BS:SW], bhc[:, 1:2],
                            r[:, BS:SW], op0=AO.add, op1=AO.mult)
                    tcin = tp.tile([128, SW], f32, tag="tcin")
                    nc.vector.tensor_tensor(tcin[:], tmpc[:],
                                            igt[:, base + 32:base + 48],
                                            op=AO.add)
                    cg = tp.tile([128, SW], f32, tag="cg")
                    nc.scalar.activation(cg[:], tcin[:], AF.Tanh)
                    # z = sigmoid(hg_z + ig_z)
                    if not OPT_IDENT:
                        nc.vector.tensor_tensor(pB[:], pB[:],
                                                igt[:, base + 16:base + 32],
                                                op=AO.add)
                    zg = tp.tile([128, SW], f32, tag="zg")
                    nc.scalar.activation(zg[:], pB[:], AF.Sigmoid)
                    oslice = oc[:, tl * SW:(tl + 1) * SW]
                    out_eng = nc.gpsimd if OPT_POOL_OUT else nc.vector
                    if OPT_ZM:
                        # h' = z*h + (1-z)*c ; z*h runs parallel to the c-path
                        zmg = tp.tile([128, SW], f32, tag="zmg")
                        nc.scalar.activation(zmg[:], pB[:], AF.Sigmoid,
                                             scale=-1.0)
                        t1 = tp.tile([128, SW], f32, tag="t1")
                        nc.vector.tensor_tensor(t1[:], zg[:], hfp_prev,
                                                op=AO.mult)
                        t2 = tp.tile([128, SW], f32, tag="t2")
                        nc.vector.tensor_tensor(t2[:], zmg[:], cg[:],
                                                op=AO.mult)
                        if SCAN_BF16:
                            hb = hbp.tile([128, SW], wdt, tag="hb")
                            nc.vector.tensor_tensor(hb[:], t1[:], t2[:],
                                                    op=AO.add)
                            out_eng.tensor_tensor(oslice, t1[:], t2[:],
                                                  op=AO.add)
                            hbf_prev = hb[:, :]
                        else:
                            nc.vector.tensor_tensor(oslice, t1[:], t2[:],
                                                    op=AO.add)
                            hbf_prev = oslice
                    else:
                        # h' = c + z * (h - c)
                        hd = tp.tile([128, SW], f32, tag="hd")
                        nc.vector.tensor_tensor(hd[:], hfp_prev, cg[:],
                                                op=AO.subtract)
                        he = tp.tile([128, SW], f32, tag="he")
                        nc.vector.tensor_tensor(he[:], zg[:], hd[:], op=AO.mult)
                        if SCAN_BF16:
                            hb = hbp.tile([128, SW], wdt, tag="hb")
                            nc.vector.tensor_tensor(hb[:], cg[:], he[:],
                                                    op=AO.add)
                            out_eng.tensor_tensor(oslice, cg[:], he[:],
                                                  op=AO.add)
                            hbf_prev = hb[:, :]
                        else:
                            nc.vector.tensor_tensor(oslice, cg[:], he[:],
                                                    op=AO.add)
                            hbf_prev = oslice
                    hfp_prev = oslice
                    for thunk in sched.get(tl, ()):
                        thunk()
                nc.sync.dma_start(out_d[:, ci * CH * SW:(ci + 1) * CH * SW],
                                  oc[:])
                if ci + 1 < nchunk:
                    igt, ig3_cur = igt_next, ig3_next
    nc.compile()
    return nc


def _get_program(t_total):
    key = (t_total, SCAN_BF16)
    if key not in _cache:
        _cache[key] = _build(t_total)
    return _cache[key]


def _prep_inputs(input_, hx, weight_ih, weight_hh, bias_ih, bias_hh, t_total):
    """Host-side shard + transpose prep.  Returns in_maps (list per core)."""
    bf = ml_dtypes.bfloat16
    wdt_np = bf if SCAN_BF16 else np.float32
    x = np.ascontiguousarray(input_[:t_total], dtype=np.float32)

    wihT = np.ascontiguousarray(weight_ih.T, dtype=np.float32)       # [128,768]
    whhT = np.ascontiguousarray(weight_hh.T).astype(wdt_np)          # [256,768]
    bsum = (bias_ih + bias_hh).astype(np.float32)
    igb = np.empty((128, 6), np.float32)
    for mi in range(4):
        igb[:, mi] = bsum[mi * 128:(mi + 1) * 128]
    igb[:, 4] = bias_ih[512:640]
    igb[:, 5] = bias_ih[640:768]
    bhc = np.stack([bias_hh[512:640], bias_hh[640:768]], axis=1)
    bhc = np.ascontiguousarray(bhc, dtype=np.float32)                # [128,2]
    bhcb = np.ascontiguousarray(np.repeat(bhc.T[:, :, None], BS, axis=2)
                                .transpose(1, 0, 2).reshape(128, SW),
                                dtype=np.float32)                    # [128,16]
    ident = np.eye(128, dtype=np.float32)

    in_maps = []
    for ci in range(NCORES):
        b0 = ci * BS
        xs = x[:, b0:b0 + BS, :]                                     # [T,BS,128]
        xT = np.ascontiguousarray(xs.transpose(2, 0, 1).reshape(IN, t_total * BS))
        h0 = hx[0, b0:b0 + BS, :].astype(np.float32)                 # [BS,256]
        h0f = np.ascontiguousarray(
            h0.T.reshape(2, 128, BS).transpose(1, 0, 2).reshape(128, SW))
        in_maps.append({
            "xT": xT,
            "wihT": wihT,
            "whhT": whhT,
            "igb": igb,
            "bhc": bhc,
            "bhcb": bhcb,
            "ident": ident,
            "h0f": h0f,
            "h0b": h0f.astype(wdt_np),
        })
    return in_maps


def _assemble(results, t_total):
    outs = []
    for ci in range(NCORES):
        o = results[ci]["outT"]                                      # [128,T*16]
        o = o.reshape(128, t_total, 2, BS).transpose(1, 3, 2, 0)
        outs.append(o.reshape(t_total, BS, H))
    output = np.ascontiguousarray(np.concatenate(outs, axis=1), dtype=np.float32)
    h_n = output[-1:, :, :].copy()
    return output, h_n


def run(inputs, t_total=T, trace=False):
    from concourse.bass_utils import run_bass_kernel_spmd
    nc = _get_program(t_total)
    in_maps = _prep_inputs(inputs["input_"], inputs["hx"], inputs["weight_ih"],
                           inputs["weight_hh"], inputs["bias_ih"],
                           inputs["bias_hh"], t_total)
    res = run_bass_kernel_spmd(nc, in_maps, core_ids=list(range(NCORES)),
                               trace=trace)
    output, h_n = _assemble(res.results, t_total)
    return (output, h_n), res


def kernel(**inputs):
    (output, h_n), _ = run(inputs)
    return output, h_n


# revision 27
# speedup vs baseline: 4206.5900x; 4206.5900x over previous
"""Trainium2 Bass kernel for a custom GRU (T=2048, B=64, IN=128, H=256).

Strategy
--------
Data-parallel over batch: 8 NeuronCores x 8 batch rows each, every core runs
the full sequential scan on its shard (the nonlinear recurrence blocks
sequence parallelism).

Per core, everything lives in a "gates/hidden on partitions" layout so the
elementwise tail uses all 128 SBUF partitions:

  phase A (per 64-step chunk): ig^T = W_ih^T.T @ x^T  (fp32 PE matmuls,
      N=512), biases folded in with a DVE tensor_scalar add, result kept in
      SBUF in the scan's column layout [128, t*48 + m*8 + b].
  scan step: 12 bf16 matmuls (W_hh^T tiles stationary, h^T moving, N=8)
      accumulate hgates into three separate PSUM banks (r / z / c) so the
      DVE/ACT tail on one bank never serializes against PE writes to the
      others.  Tail: sigmoid/tanh on ACT (one table set), fused
      (psum+bias)*r via scalar_tensor_tensor on DVE, h update on DVE.
      h is kept fp32 for the update/output; only the matmul operand is
      rounded to bf16 (measured end-to-end rel_l2 ~6.7e-4 vs fp32).

Host side does all transposes (numpy), so the device never needs a
transpose: inputs are shipped pre-transposed, the output is shipped back in
[128, T*16] layout and rearranged to [T, B, H] on the host.
"""

import sys

if "/opt/trn_rl_repo" not in sys.path:
    sys.path.insert(0, "/opt/trn_rl_repo")

import numpy as np
import ml_dtypes

T, B, IN, H = 2048, 64, 128, 256
G3 = 3 * H               # 768
NCORES = 8
BS = B // NCORES         # 8 batch rows per core
SW = 2 * BS              # 16 columns of h per timestep (2 blocks of 128 dims)
CH = 64                  # timesteps per chunk
SCAN_BF16 = True         # bf16 recurrent matmul (fp32 psum accumulate)
OPT_IDENT = True         # preload psum with ig_r/ig_z/bhh_c via identity matmul
OPT_POOL_OUT = True      # fp32 output copy on gpsimd instead of DVE
OPT_ZM = True            # h' = z*h + (1-z)*c with 1-z = sigmoid(-x); z*h off-chain
PHASEA_F32R = True      # igates matmul in float32r (fast fp32 PE mode)

_cache = {}


def _build(t_total):
    import concourse.bacc as bacc
    import concourse.mybir as mybir
    import concourse.tile as tile

    f32 = mybir.dt.float32
    xdt = mybir.dt.float32r if PHASEA_F32R else f32
    wdt = mybir.dt.bfloat16 if SCAN_BF16 else f32
    AO = mybir.AluOpType
    AF = mybir.ActivationFunctionType

    nchunk = t_total // CH
    nc = bacc.Bacc("TRN2", target_bir_lowering=False, debug=False,
                   num_devices=NCORES)

    xT_d = nc.dram_tensor("xT", [IN, t_total * BS], xdt, kind="ExternalInput")
    wihT_d = nc.dram_tensor("wihT", [IN, G3], xdt, kind="ExternalInput")
    whhT_d = nc.dram_tensor("whhT", [H, G3], wdt, kind="ExternalInput")
    igb_d = nc.dram_tensor("igb", [128, 6], f32, kind="ExternalInput")
    bhc_d = nc.dram_tensor("bhc", [128, 2], f32, kind="ExternalInput")
    bhcb_d = nc.dram_tensor("bhcb", [128, SW], f32, kind="ExternalInput")
    ident_d = nc.dram_tensor("ident", [128, 128], f32, kind="ExternalInput")
    h0f_d = nc.dram_tensor("h0f", [128, SW], f32, kind="ExternalInput")
    h0b_d = nc.dram_tensor("h0b", [128, SW], wdt, kind="ExternalInput")
    out_d = nc.dram_tensor("outT", [128, t_total * SW], f32,
                           kind="ExternalOutput")

    with tile.TileContext(nc) as tc:
        with (
            tc.tile_pool(name="consts", bufs=1) as consts,
            tc.tile_pool(name="xc", bufs=3) as xcp,
            tc.tile_pool(name="ig", bufs=2) as igp,
            tc.tile_pool(name="outc", bufs=2) as outp,
            tc.tile_pool(name="tails", bufs=3) as tp,
            tc.tile_pool(name="hbf", bufs=3) as hbp,
            tc.tile_pool(name="psA", bufs=2, space="PSUM") as psa,
            tc.tile_pool(name="psB", bufs=2, space="PSUM") as psb,
            tc.tile_pool(name="psC", bufs=2, space="PSUM") as psc,
            tc.tile_pool(name="psP", bufs=2, space="PSUM") as psp,
        ):
            wih = consts.tile([IN, G3], xdt, tag="wih")
            nc.sync.dma_start(wih[:], wihT_d[:])
            whh0 = consts.tile([128, G3], wdt, tag="whh0")
            whh1 = consts.tile([128, G3], wdt, tag="whh1")
            nc.sync.dma_start(whh0[:], whhT_d[0:128, :])
            nc.sync.dma_start(whh1[:], whhT_d[128:256, :])
            igb = consts.tile([128, 6], f32, tag="igb")
            nc.sync.dma_start(igb[:], igb_d[:])
            bhc = consts.tile([128, 2], f32, tag="bhc")
            nc.sync.dma_start(bhc[:], bhc_d[:])
            if OPT_IDENT:
                bhcb = consts.tile([128, SW], f32, tag="bhcb")
                nc.sync.dma_start(bhcb[:], bhcb_d[:])
                ident = consts.tile([128, 128], f32, tag="ident")
                nc.sync.dma_start(ident[:], ident_d[:])
            h0f = consts.tile([128, SW], f32, tag="h0f")
            nc.sync.dma_start(h0f[:], h0f_d[:])
            h0b = consts.tile([128, SW], wdt, tag="h0b")
            nc.sync.dma_start(h0b[:], h0b_d[:])

            mm = nc.tensor.matmul
            hbf_prev = h0b[:, :]     # bf16 (or fp32) h for matmul rhs
            hfp_prev = h0f[:, :]     # fp32 h for the update

            NP = 1              # bias-add split (keep DVE pieces small)

            def phase_a_mm(xc, m):
                """One M-tile of the igates precompute matmul."""
                pp = psp.tile([128, CH * BS], f32, tag="pp")
                mm(pp[:], wih[:, m * 128:(m + 1) * 128], xc[:],
                   start=True, stop=True)
                return pp

            def phase_a_bias(pp, ig3, m, j, on_act=False):
                """Bias add + psum->sbuf rearrange, one slice of timesteps."""
                pp3 = pp[:].rearrange("p (t b) -> p t b", b=BS)
                t0, t1 = j * (CH // NP), (j + 1) * (CH // NP)
                dst = ig3[:, t0:t1, m * BS:(m + 1) * BS]
                src = pp3[:, t0:t1, :]
                if on_act:
                    nc.scalar.activation(dst, src, AF.Identity,
                                         bias=igb[:, m:m + 1])
                else:
                    nc.vector.tensor_scalar_add(dst, src, igb[:, m:m + 1])

            def load_chunk(ci):
                xc = xcp.tile([IN, CH * BS], xdt, tag="xc")
                nc.sync.dma_start(xc[:],
                                  xT_d[:, ci * CH * BS:(ci + 1) * CH * BS])
                igt = igp.tile([128, CH * 48], f32, tag="ig")
                ig3 = igt[:].rearrange("p (t c) -> p t c", c=48)
                return xc, igt, ig3

            # chunk 0's igates run up front; each later chunk's igates are
            # interleaved into the previous chunk's scan (PE/DVE idle slots)
            xc_cur, igt, ig3_cur = load_chunk(0)
            for m in range(6):
                pp = phase_a_mm(xc_cur, m)
                for j in range(NP):
                    phase_a_bias(pp, ig3_cur, m, j)

            for ci in range(nchunk):
                if ci + 1 < nchunk:
                    xc_next, igt_next, ig3_next = load_chunk(ci + 1)
                    # emission schedule: step -> list of thunks
                    sched = {}
                    pp_of = {}
                    for m in range(6):
                        def mk_mm(m=m):
                            pp_of[m] = phase_a_mm(xc_next, m)
                        sched.setdefault(8 * (m + 1), []).append(mk_mm)
                        for j in range(NP):
                            def mk_bias(m=m, j=j):
                                phase_a_bias(pp_of[m], ig3_next, m, j,
                                             on_act=(m % 2 == 1))
                            sched.setdefault(8 * (m + 1) + 2 * j + 1,
                                             []).append(mk_bias)
                else:
                    sched = {}

                # ---- scan over the chunk ----
                oc = outp.tile([128, CH * SW], f32, tag="oc")
                for tl in range(CH):
                    base = tl * 48
                    pA = psa.tile([128, SW], f32, tag="pA")
                    pB = psb.tile([128, SW], f32, tag="pB")
                    pC = psc.tile([128, SW], f32, tag="pC")
                    hb0 = hbf_prev[:, 0:BS]
                    hb1 = hbf_prev[:, BS:SW]
                    if OPT_IDENT:
                        # preload psum banks via identity matmul so the
                        # weight matmuls accumulate straight onto ig / bias
                        mm(pA[:], ident[:], igt[:, base:base + 16],
                           start=True, stop=False)
                        mm(pB[:], ident[:], igt[:, base + 16:base + 32],
                           start=True, stop=False)
                        mm(pC[:], ident[:], bhcb[:], start=True, stop=False)
                        s0 = False
                    else:
                        s0 = True
                    # r gates first (c-path needs r), then c, then z
                    mm(pA[:, 0:BS], whh0[:, 0:128], hb0, start=s0, stop=False)
                    mm(pA[:, 0:BS], whh1[:, 0:128], hb1, start=False, stop=False)
                    mm(pA[:, BS:SW], whh0[:, 128:256], hb0, start=False, stop=False)
                    mm(pA[:, BS:SW], whh1[:, 128:256], hb1, start=False, stop=True)
                    mm(pC[:, 0:BS], whh0[:, 512:640], hb0, start=s0, stop=False)
                    mm(pC[:, 0:BS], whh1[:, 512:640], hb1, start=False, stop=False)
                    mm(pC[:, BS:SW], whh0[:, 640:768], hb0, start=False, stop=False)
                    mm(pC[:, BS:SW], whh1[:, 640:768], hb1, start=False, stop=True)
                    mm(pB[:, 0:BS], whh0[:, 256:384], hb0, start=s0, stop=False)
                    mm(pB[:, 0:BS], whh1[:, 256:384], hb1, start=False, stop=False)
                    mm(pB[:, BS:SW], whh0[:, 384:512], hb0, start=False, stop=False)
                    mm(pB[:, BS:SW], whh1[:, 384:512], hb1, start=False, stop=True)

                    r = tp.tile([128, SW], f32, tag="r")
                    if OPT_IDENT:
                        # pA already holds ig_r + hg_r
                        nc.scalar.activation(r[:], pA[:], AF.Sigmoid)
                        tmpc = tp.tile([128, SW], f32, tag="tmpc")
                        nc.vector.tensor_tensor(tmpc[:], pC[:], r[:],
                                                op=AO.mult)
                    else:
                        nc.vector.tensor_tensor(pA[:], pA[:],
                                                igt[:, base:base + 16],
                                                op=AO.add)
                        nc.scalar.activation(r[:], pA[:], AF.Sigmoid)
                        # tmpc = (hg_c + b_hh_c) * r   (per-128-block bias)
                        tmpc = tp.tile([128, SW], f32, tag="tmpc")
                        nc.vector.scalar_tensor_tensor(
                            tmpc[:, 0:BS], pC[:, 0:BS], bhc[:, 0:1], r[:, 0:BS],
                            op0=AO.add, op1=AO.mult)
                        nc.vector.scalar_tensor_tensor(
                            tmpc[:, BS:SW], pC[:, BS:SW], bhc[:, 1:2],
                            r[:, BS:SW], op0=AO.add, op1=AO.mult)
                    tcin = tp.tile([128, SW], f32, tag="tcin")
                    nc.vector.tensor_tensor(tcin[:], tmpc[:],
                                            igt[:, base + 32:base + 48],
                                            op=AO.add)
                    cg = tp.tile([128, SW], f32, tag="cg")
                    nc.scalar.activation(cg[:], tcin[:], AF.Tanh)
                    # z = sigmoid(hg_z + ig_z)
                    if not OPT_IDENT:
                        nc.vector.tensor_tensor(pB[:], pB[:],
                                                igt[:, base + 16:base + 32],
                                                op=AO.add)
                    zg = tp.tile([128, SW], f32, tag="zg")
                    nc.scalar.activation(zg[:], pB[:], AF.Sigmoid)
                    oslice = oc[:, tl * SW:(tl + 1) * SW]
                    out_eng = nc.gpsimd if OPT_POOL_OUT else nc.vector
                    if OPT_ZM:
                        # h' = z*h + (1-z)*c ; z*h runs parallel to the c-path
                        zmg = tp.tile([128, SW], f32, tag="zmg")
                        nc.scalar.activation(zmg[:], pB[:], AF.Sigmoid,
                                             scale=-1.0)
                        t1 = tp.tile([128, SW], f32, tag="t1")
                        nc.vector.tensor_tensor(t1[:], zg[:], hfp_prev,
                                                op=AO.mult)
                        t2 = tp.tile([128, SW], f32, tag="t2")
                        nc.vector.tensor_tensor(t2[:], zmg[:], cg[:],
                                                op=AO.mult)
                        if SCAN_BF16:
                            hb = hbp.tile([128, SW], wdt, tag="hb")
                            nc.vector.tensor_tensor(hb[:], t1[:], t2[:],
                                                    op=AO.add)
                            out_eng.tensor_tensor(oslice, t1[:], t2[:],
                                                  op=AO.add)
                            hbf_prev = hb[:, :]
                        else:
                            nc.vector.tensor_tensor(oslice, t1[:], t2[:],
                                                    op=AO.add)
                            hbf_prev = oslice
                    else:
                        # h' = c + z * (h - c)
                        hd = tp.tile([128, SW], f32, tag="hd")
                        nc.vector.tensor_tensor(hd[:], hfp_prev, cg[:],
                                                op=AO.subtract)
                        he = tp.tile([128, SW], f32, tag="he")
                        nc.vector.tensor_tensor(he[:], zg[:], hd[:], op=AO.mult)
                        if SCAN_BF16:
                            hb = hbp.tile([128, SW], wdt, tag="hb")
                            nc.vector.tensor_tensor(hb[:], cg[:], he[:],
                                                    op=AO.add)
                            out_eng.tensor_tensor(oslice, cg[:], he[:],
                                                  op=AO.add)
                            hbf_prev = hb[:, :]
                        else:
                            nc.vector.tensor_tensor(oslice, cg[:], he[:],
                                                    op=AO.add)
                            hbf_prev = oslice
                    hfp_prev = oslice
                    for thunk in sched.get(tl, ()):
                        thunk()
                nc.sync.dma_start(out_d[:, ci * CH * SW:(ci + 1) * CH * SW],
                                  oc[:])
                if ci + 1 < nchunk:
                    igt, ig3_cur = igt_next, ig3_next
    nc.compile()
    return nc


def _get_program(t_total):
    key = (t_total, SCAN_BF16)
    if key not in _cache:
        _cache[key] = _build(t_total)
    return _cache[key]


def _prep_inputs(input_, hx, weight_ih, weight_hh, bias_ih, bias_hh, t_total):
    """Host-side shard + transpose prep.  Returns in_maps (list per core)."""
    bf = ml_dtypes.bfloat16
    wdt_np = bf if SCAN_BF16 else np.float32
    x = np.ascontiguousarray(input_[:t_total], dtype=np.float32)

    wihT = np.ascontiguousarray(weight_ih.T, dtype=np.float32)       # [128,768]
    whhT = np.ascontiguousarray(weight_hh.T).astype(wdt_np)          # [256,768]
    bsum = (bias_ih + bias_hh).astype(np.float32)
    igb = np.empty((128, 6), np.float32)
    for mi in range(4):
        igb[:, mi] = bsum[mi * 128:(mi + 1) * 128]
    igb[:, 4] = bias_ih[512:640]
    igb[:, 5] = bias_ih[640:768]
    bhc = np.stack([bias_hh[512:640], bias_hh[640:768]], axis=1)
    bhc = np.ascontiguousarray(bhc, dtype=np.float32)                # [128,2]
    bhcb = np.ascontiguousarray(np.repeat(bhc.T[:, :, None], BS, axis=2)
                                .transpose(1, 0, 2).reshape(128, SW),
                                dtype=np.float32)                    # [128,16]
    ident = np.eye(128, dtype=np.float32)

    in_maps = []
    for ci in range(NCORES):
        b0 = ci * BS
        xs = x[:, b0:b0 + BS, :]                                     # [T,BS,128]
        xT = np.ascontiguousarray(xs.transpose(2, 0, 1).reshape(IN, t_total * BS))
        h0 = hx[0, b0:b0 + BS, :].astype(np.float32)                 # [BS,256]
        h0f = np.ascontiguousarray(
            h0.T.reshape(2, 128, BS).transpose(1, 0, 2).reshape(128, SW))
        in_maps.append({
            "xT": xT,
            "wihT": wihT,
            "whhT": whhT,
            "igb": igb,
            "bhc": bhc,
            "bhcb": bhcb,
            "ident": ident,
            "h0f": h0f,
            "h0b": h0f.astype(wdt_np),
        })
    return in_maps


def _assemble(results, t_total):
    outs = []
    for ci in range(NCORES):
        o = results[ci]["outT"]                                      # [128,T*16]
        o = o.reshape(128, t_total, 2, BS).transpose(1, 3, 2, 0)
        outs.append(o.reshape(t_total, BS, H))
    output = np.ascontiguousarray(np.concatenate(outs, axis=1), dtype=np.float32)
    h_n = output[-1:, :, :].copy()
    return output, h_n


def run(inputs, t_total=T, trace=False):
    from concourse.bass_utils import run_bass_kernel_spmd
    nc = _get_program(t_total)
    in_maps = _prep_inputs(inputs["input_"], inputs["hx"], inputs["weight_ih"],
                           inputs["weight_hh"], inputs["bias_ih"],
                           inputs["bias_hh"], t_total)
    res = run_bass_kernel_spmd(nc, in_maps, core_ids=list(range(NCORES)),
                               trace=trace)
    output, h_n = _assemble(res.results, t_total)
    return (output, h_n), res


def kernel(**inputs):
    (output, h_n), _ = run(inputs)
    return output, h_n
